# revision 14
# baseline (speedup 1.0000x reference)
"""DynamicConv2D Trainium2 kernel (8-core SPMD, data-parallel over batch).

Per sample: GAP -> MLP -> softmax routing over K=4 kernel banks, weight-space
aggregation, then a 3x3 SAME conv with the per-sample aggregated kernel.

Device strategy (per core, 4 samples processed as 2 stacked PAIRS):
  - Host packs x TRANSPOSED and width-padded bf16 [128, SP] per PAIR:
    partitions 0:64 = sample A channels, 64:128 = sample B channels.
    No shifted copy -- input DMA bytes are halved vs the paired-tap layout.
  - Conv uses the PE's 64x64 quadrant tiling: 4 independent tiles
    (rowgrp = sample, colgrp = image half) stream concurrently, 9 K=64
    taps each, accumulating in PSUM.  That is ~100% MAC utilization
    (the K=128 paired-tap scheme wastes 1/3 on zero rows).
  - The PE queue serializes LDWEIGHTS at ~108 ns each, so each weight
    load is amortized over two N=512 matmuls ([128,1024] two-bank PSUM
    supergroups); a post-compile pass deletes the redundant LDWEIGHTS
    the legalizer inserts (hardware keeps tile weights across matmuls).
    The 8320-col half splits into 7x1024 + 1x1152(512+512+128); the 128
    tail rides the MLP's PSUM banks so it adds no extra weight loads.
  - Routing MLP: layer-1 for sample B uses tile (64,0) (SBUF 64:128 ->
    PSUM 0:64), so BOTH chains continue on partitions 0:64 and the
    softmax/pi broadcast is a plain partition-0 broadcast.  pi is
    normalized on-chip (1/sum folded before W_agg), drains are bias-only.
  - Engine split: SP queue = input loads; ACT = PSUM drains (+bias) and
    output stores; DVE = pooled partials + routing chain + W_agg FMAs;
    GpSimd = halo memsets + pi broadcast; PE = conv + MLP matmuls.
  - Software pipelining: routing chain for pair s+1 is emitted between
    the first supergroups of pair s's conv; loads run two pairs ahead.
"""

import numpy as np
import ml_dtypes

BF16 = ml_dtypes.bfloat16

B, H, W, C, F = 32, 128, 128, 64, 64
KK, HID = 4, 16
TAPS = 9
TEMP = 30.0
NCORES, BPC = 8, 4
NPAIR = BPC // 2    # 2 pairs per core
WP = W + 2          # padded width (zero col at w'=0 and w'=129)
SP = H * WP         # 16640 padded spatial per sample
PADL = 136          # SBUF zero halo before the image (taps read to -WP-1)
PADR = 136          # SBUF zero halo after (taps read up to +WP+1)
HALF = SP // 2      # 8320, image halves (rows <64 / >=64)
CHW = SP // 4       # input DMA chunk width (4160 cols, ~1.06 MB)
SF = TAPS * F       # 576 weight cols per sample half

# supergroups over one half: (col offset, [(block off, block width), ...])
SGS = [(g * 1024, [(0, 512), (512, 512)]) for g in range(7)]
SGS.append((7168, [(0, 512), (512, 512), (1024, 128)]))

_CACHE = {}


def _ldw_key(ins_ap):
    """Stable identity key for a lowered weights AP."""
    return repr(ins_ap)


def _dedup_ldweights(nc):
    """Remove InstLdweights that reload the exact weights already resident
    in the same PE tile.  The legalizer inserts one load per matmul; the
    hardware keeps a tile's stationary weights across matmuls, so a
    second load of the same AP into the same tile is pure queue time
    (~108 ns each).  Only sync-free loads are removed; any matmul whose
    weights AP differs from the tile's resident key invalidates it."""
    import concourse.mybir as mybir

    removed = 0
    for blk in nc.main_func.blocks:
        last = {}
        keep = []
        for ins in blk.instructions:
            if isinstance(ins, mybir.InstLdweights):
                tp = tuple(ins.tile_position or (0, 0))
                key = _ldw_key(ins.ins[0])
                si = getattr(ins, "sync_info", None)
                clean = si is None or (
                    not getattr(si, "on_wait", None)
                    and not getattr(si, "on_update", None)
                )
                if clean and last.get(tp) == key:
                    removed += 1
                    continue
                last[tp] = key
            elif isinstance(ins, mybir.InstMatmult):
                tp = tuple(ins.tile_position or (0, 0))
                wkey = _ldw_key(ins.ins[1])
                if last.get(tp) != wkey:
                    # self/alternate load (e.g. fp32 MLP matmul) clobbers
                    last[tp] = None
            keep.append(ins)
        blk.instructions[:] = keep
    return removed


def _build_program(dbg=False, reps=1):
    import concourse.bacc as bacc
    import concourse.mybir as mybir
    import concourse.tile as tile

    f32 = mybir.dt.float32
    bf16 = mybir.dt.bfloat16
    AX = mybir.AxisListType.X
    ALU = mybir.AluOpType

    nc = bacc.Bacc("TRN2", target_bir_lowering=False, debug=False)

    x2_d = nc.dram_tensor("x2", [NPAIR, 128, SP], bf16, kind="ExternalInput")
    wk_d = nc.dram_tensor("wk", [128, KK * SF], f32, kind="ExternalInput")
    w1s_d = nc.dram_tensor("w1s", [128, 64], f32, kind="ExternalInput")
    b1_d = nc.dram_tensor("b1", [HID, 1], f32, kind="ExternalInput")
    w2s_d = nc.dram_tensor("w2s", [64, KK], f32, kind="ExternalInput")
    b2_d = nc.dram_tensor("b2", [1, KK], f32, kind="ExternalInput")
    bkt_d = nc.dram_tensor("bkt", [128, KK], f32, kind="ExternalInput")
    yp_d = nc.dram_tensor("ypad", [BPC, 128, HALF], bf16,
                          kind="ExternalOutput")
    if dbg:
        dpib_d = nc.dram_tensor("dpib", [128, 2 * KK], f32,
                                kind="ExternalOutput")
        dwg_d = nc.dram_tensor("dwg", [NPAIR, 128, SF], bf16,
                               kind="ExternalOutput")

    with tile.TileContext(nc) as tc:
        from contextlib import ExitStack
        with ExitStack() as ctx:
            # PSUM: four static [128,1024] two-bank tiles (all 8 banks).
            # A-supergroups ping-pong pc0/pc1, B-supergroups pc2/pc3, so
            # every conv bank has a full 2-supergroup drain window.  The
            # MLP tile and the 128-col tail borrow pc0 during its idle
            # windows (mid-sg1 / sg7).
            psp = ctx.enter_context(tc.tile_pool(name="psp", bufs=1,
                                                 space="PSUM"))
            cst = ctx.enter_context(tc.tile_pool(name="cst", bufs=1))
            xtp = ctx.enter_context(tc.tile_pool(name="xtp", bufs=4))
            ytp = ctx.enter_context(tc.tile_pool(name="ytp", bufs=3))
            wgp = ctx.enter_context(tc.tile_pool(name="wgp", bufs=2))
            smp = ctx.enter_context(tc.tile_pool(name="smp", bufs=2))

            wk_t = cst.tile([128, KK * SF], f32)
            w1s_t = cst.tile([128, 64], f32)
            b1_t = cst.tile([HID, 1], f32)
            w2s_t = cst.tile([64, KK], f32)
            b2_t = cst.tile([1, KK], f32)
            bkt_t = cst.tile([128, KK], f32)
            bagg_t = cst.tile([128, BPC], f32)
            hbuf_t = cst.tile([128, 128], f32)  # col 0 = hA, col 64 = hB

            def emit_consts():
                nc.gpsimd.memset(hbuf_t[:], 0.0)
                nc.sync.dma_start(w1s_t[:], w1s_d.ap())
                nc.sync.dma_start(b1_t[:], b1_d.ap())
                nc.sync.dma_start(w2s_t[:], w2s_d.ap())
                nc.sync.dma_start(b2_t[:], b2_d.ap())
                nc.sync.dma_start(wk_t[:], wk_d.ap())
                nc.sync.dma_start(bkt_t[:], bkt_d.ap())

            def emit_load(p):
                """Halo memsets + input chunk DMAs + pooled partials.
                The reduces sit ADJACENT to their dma_starts so the
                framework binds their waits to the right chunk
                completions; the 4-deep xt ring means the chunks land a
                full pair early, so these reduces drain off the DVE
                queue before the next routing chain is emitted."""
                xt = xtp.tile([128, PADL + SP + PADR], bf16, tag="xt")
                nc.gpsimd.memset(xt[:, 0:PADL], 0.0)
                nc.gpsimd.memset(xt[:, PADL + SP:PADL + SP + PADR], 0.0)
                for i in range(4):
                    o = i * CHW
                    nc.sync.dma_start(xt[:, PADL + o:PADL + o + CHW],
                                      x2_d.ap()[p][:, o:o + CHW])
                pp = smp.tile([128, 4], f32, tag="pp")
                for i in range(4):
                    nc.vector.reduce_sum(
                        pp[:, i:i + 1],
                        xt[:, PADL + i * CHW:PADL + (i + 1) * CHW],
                        axis=AX)
                return xt, pp

            def emit_chain_mm(pp):
                """Routing part A: pooled combine -> MLP, in a 2-bank
                pc0 tile borrowed mid-sg1 (pc0 idles then).  hps/lps for
                A live in bank a, for B in bank b; sample B's layer-1
                uses tile (64,0) so both chains continue on partitions
                0:64."""
                pl = smp.tile([128, 1], f32, tag="pl")
                nc.vector.reduce_sum(pl[:], pp[:], axis=AX)
                mm = psp.tile([128, 1024], f32, tag="pc0")
                nc.tensor.matmul(mm[0:64, 0:1], lhsT=w1s_t[0:64, :],
                                 rhs=pl[0:64, 0:1], start=True, stop=True,
                                 tile_position=(0, 0))
                nc.tensor.matmul(mm[0:64, 512:513], lhsT=w1s_t[64:128, :],
                                 rhs=pl[64:128, 0:1], start=True, stop=True,
                                 tile_position=(64, 0))
                # relu(h + b1) into hbuf cols 0 (A) and 64 (B)
                nc.vector.tensor_scalar(hbuf_t[0:HID, 0:1],
                                        mm[0:HID, 0:1],
                                        scalar1=b1_t[:, 0:1], scalar2=0.0,
                                        op0=ALU.add, op1=ALU.max)
                nc.vector.tensor_scalar(hbuf_t[0:HID, 64:65],
                                        mm[0:HID, 512:513],
                                        scalar1=b1_t[:, 0:1], scalar2=0.0,
                                        op0=ALU.add, op1=ALU.max)
                nc.tensor.matmul(mm[0:64, 4:4 + KK],
                                 lhsT=hbuf_t[0:64, 0:64],
                                 rhs=w2s_t[:], start=True, stop=True,
                                 tile_position=(0, 0))
                nc.tensor.matmul(mm[0:64, 516:516 + KK],
                                 lhsT=hbuf_t[0:64, 64:128],
                                 rhs=w2s_t[:], start=True, stop=True,
                                 tile_position=(0, 0))
                return mm

            def emit_chain_post(s, mm):
                """Routing part B (DVE/GpSimd): softmax via 3rd-order
                Taylor exp (|logits/T| ~ 1e-3), normalized on-chip, then
                W_agg for both samples and the bias columns."""
                exb = smp.tile([1, 2 * KK], f32, tag="exb")
                for half in range(2):
                    lps = mm[0:1, 512 * half + 4:512 * half + 4 + KK]
                    lg = smp.tile([1, KK], f32, tag=f"lg{half}")
                    nc.vector.tensor_tensor(lg[:], lps, b2_t[:],
                                            op=ALU.add)
                    # exp(z) ~= 1 + z + z^2/2
                    e1 = smp.tile([1, KK], f32, tag=f"e1{half}")
                    nc.vector.tensor_scalar(e1[:], lg[:], scalar1=0.5,
                                            scalar2=1.0, op0=ALU.mult,
                                            op1=ALU.add)
                    nc.vector.tensor_tensor(e1[:], e1[:], lg[:],
                                            op=ALU.mult)
                    ex = smp.tile([1, KK], f32, tag=f"ex{half}")
                    nc.vector.tensor_scalar(ex[:], e1[:], scalar1=1.0,
                                            scalar2=None, op0=ALU.add)
                    sm = smp.tile([1, 1], f32, tag=f"sm{half}")
                    nc.vector.reduce_sum(sm[:], ex[:], axis=AX)
                    rc = smp.tile([1, 1], f32, tag=f"rc{half}")
                    nc.vector.reciprocal(rc[:], sm[:])
                    nc.vector.tensor_scalar(
                        exb[:, half * KK:(half + 1) * KK], ex[:],
                        scalar1=rc[0:1, 0:1], scalar2=None, op0=ALU.mult)
                pib = smp.tile([128, 2 * KK], f32, tag="pib")
                nc.gpsimd.partition_broadcast(pib[:], exb[:])
                # per-half pi columns: rows 0:64 <- A's pi, 64:128 <- B's
                pim = smp.tile([128, KK], f32, tag="pim")
                nc.vector.tensor_scalar(pim[0:64, :], pib[0:64, 0:KK],
                                        scalar1=0.0, scalar2=None,
                                        op0=ALU.add)
                nc.vector.tensor_scalar(pim[64:128, :], pib[64:128, KK:],
                                        scalar1=0.0, scalar2=None,
                                        op0=ALU.add)
                # W_agg = sum_k pi_k * Wk for both halves at once
                acc = wgp.tile([128, SF], f32, tag="acc")
                nc.vector.tensor_scalar(acc[:], wk_t[:, 0:SF],
                                        scalar1=pim[:, 0:1], scalar2=None,
                                        op0=ALU.mult)
                for k in range(1, KK - 1):
                    nc.vector.scalar_tensor_tensor(
                        acc[:], wk_t[:, k * SF:(k + 1) * SF],
                        pim[:, k:k + 1], acc[:], op0=ALU.mult, op1=ALU.add)
                wg = wgp.tile([128, SF], bf16, tag="wg")
                nc.vector.scalar_tensor_tensor(
                    wg[:], wk_t[:, (KK - 1) * SF:KK * SF],
                    pim[:, KK - 1:KK], acc[:], op0=ALU.mult, op1=ALU.add)
                # bias columns (pi already normalized): bagg[:, sample]
                for half in range(2):
                    ca = (2 * (s % NPAIR) + half)
                    bu = smp.tile([128, 1], f32, tag=f"bu{half}")
                    nc.vector.tensor_scalar(
                        bu[:], bkt_t[:, 0:1],
                        scalar1=pib[:, half * KK:half * KK + 1],
                        scalar2=None, op0=ALU.mult)
                    for k in range(1, KK - 1):
                        nc.vector.scalar_tensor_tensor(
                            bu[:], bkt_t[:, k:k + 1],
                            pib[:, half * KK + k:half * KK + k + 1], bu[:],
                            op0=ALU.mult, op1=ALU.add)
                    nc.vector.scalar_tensor_tensor(
                        bagg_t[:, ca:ca + 1], bkt_t[:, KK - 1:KK],
                        pib[:, half * KK + KK - 1:half * KK + KK], bu[:],
                        op0=ALU.mult, op1=ALU.add)
                if dbg:
                    nc.sync.dma_start(dpib_d.ap(), pib[:])
                    nc.sync.dma_start(dwg_d.ap()[s % NPAIR], wg[:])
                return wg

            def emit_sg(s, g, xt, wg, ytA, ytB, o, blocks, tap_hooks):
                """One supergroup: 9 taps x blocks, 4 quadrant tiles.
                Each tile's weights are loaded once per tap (the dedup
                pass removes the per-matmul reloads for blocks > 0).
                The 128-col tail (sg7 3rd block) accumulates in a
                borrowed pc0 tile: tailA in its bank a, tailB in bank b."""
                psA = psp.tile([128, 1024], f32, tag=f"pc{g % 2}")
                psB = psp.tile([128, 1024], f32, tag=f"pc{2 + g % 2}")
                tt = None
                if blocks[-1][0] == 1024:
                    tt = psp.tile([128, 1024], f32, tag="pc0")
                for j in range(TAPS):
                    off = (j // 3 - 1) * WP + (j % 3 - 1)
                    st = j == 0
                    sp = j == TAPS - 1
                    lA = wg[0:64, j * F:(j + 1) * F]
                    lB = wg[64:128, j * F:(j + 1) * F]
                    for (b, w) in blocks:
                        if b < 1024:
                            aA = psA[0:64, b:b + w]
                            aA2 = psA[64:128, b:b + w]
                            aB = psB[0:64, b:b + w]
                            aB2 = psB[64:128, b:b + w]
                        else:
                            aA = tt[0:64, 0:w]
                            aA2 = tt[64:128, 0:w]
                            aB = tt[0:64, 512:512 + w]
                            aB2 = tt[64:128, 512:512 + w]
                        base0 = PADL + o + b + off
                        base1 = base0 + HALF
                        nc.tensor.matmul(aA, lhsT=lA,
                                         rhs=xt[0:64, base0:base0 + w],
                                         start=st, stop=sp,
                                         tile_position=(0, 0))
                        nc.tensor.matmul(aA2, lhsT=lA,
                                         rhs=xt[0:64, base1:base1 + w],
                                         start=st, stop=sp,
                                         tile_position=(0, 64))
                        nc.tensor.matmul(aB, lhsT=lB,
                                         rhs=xt[64:128, base0:base0 + w],
                                         start=st, stop=sp,
                                         tile_position=(64, 0))
                        nc.tensor.matmul(aB2, lhsT=lB,
                                         rhs=xt[64:128, base1:base1 + w],
                                         start=st, stop=sp,
                                         tile_position=(64, 64))
                    hook = tap_hooks.get(j)
                    if hook is not None:
                        hook()
                gw = sum(w for (b, w) in blocks if b < 1024)
                caA = 2 * (s % NPAIR)
                ident = mybir.ActivationFunctionType.Identity
                if tt is not None:
                    # tail drains first: pc0 is the first tile the next
                    # pair's sg0 reuses
                    tw = blocks[-1][1]
                    to = o + blocks[-1][0]
                    nc.scalar.activation(ytA[:, to:to + tw], tt[:, 0:tw],
                                         ident,
                                         bias=bagg_t[:, caA:caA + 1])
                    nc.scalar.activation(ytB[:, to:to + tw],
                                         tt[:, 512:512 + tw], ident,
                                         bias=bagg_t[:, caA + 1:caA + 2])
                nc.scalar.activation(ytA[:, o:o + gw], psA[:, 0:gw], ident,
                                     bias=bagg_t[:, caA:caA + 1])
                nc.scalar.activation(ytB[:, o:o + gw], psB[:, 0:gw], ident,
                                     bias=bagg_t[:, caA + 1:caA + 2])

            def emit_conv(s, xt, wg, hooks, tap_hooks_by_sg):
                """Full conv for one pair; hooks[g] emitted after
                supergroup g, tap_hooks_by_sg[g][j] after tap j of
                supergroup g (pipelined routing for the next pair)."""
                ytA = ytp.tile([128, HALF], bf16, tag="yt")
                ytB = ytp.tile([128, HALF], bf16, tag="yt")
                for g, (o, blocks) in enumerate(SGS):
                    emit_sg(s, g, xt, wg, ytA, ytB, o, blocks,
                            tap_hooks_by_sg.get(g, {}))
                    hook = hooks.get(g)
                    if hook is not None:
                        hook()
                pr = s % NPAIR
                nc.scalar.dma_start(yp_d.ap()[2 * pr], ytA[:])
                nc.scalar.dma_start(yp_d.ap()[2 * pr + 1], ytB[:])

            # ---- software-pipelined main loop over pairs ----
            # per conv(s):  pre-sg0: DMA issue for pair s+2
            #               sg1 tap2/tap4: routing chain for pair s+1
            #               after sg5: pooled partials for pair s+2
            S = reps * NPAIR
            ld, ch = {}, {}
            ld[0] = emit_load(0)
            emit_consts()
            ch[0] = emit_chain_post(0, emit_chain_mm(ld[0][1]))
            if S > 1:
                ld[1] = emit_load(1)
            for s in range(S):
                if s + 2 < S:
                    ld[s + 2] = emit_load((s + 2) % NPAIR)
                xt, _ = ld.pop(s)
                wg = ch.pop(s)
                tap_hooks = {}
                if s + 1 < S:
                    box = {}

                    def hmm(box=box, s=s):
                        box["mm"] = emit_chain_mm(ld[s + 1][1])

                    def hpost(box=box, s=s):
                        ch[s + 1] = emit_chain_post(s + 1, box["mm"])

                    tap_hooks[1] = {2: hmm, 4: hpost}
                emit_conv(s, xt, wg, {}, tap_hooks)

    nc.compile()
    n = _dedup_ldweights(nc)
    assert n > 0, "LDWEIGHTS dedup removed nothing -- emission changed?"
    return nc


def _get_program():
    if "nc" not in _CACHE:
        _CACHE["nc"] = _build_program()
    return _CACHE["nc"]


def _host_pack_x(x):
    # [B, H, W, C] fp32 -> [B//2 pairs, 128, SP] bf16 per core slice:
    # width-padded, transposed to [c, spatial]; partitions 0:64 = even
    # sample, 64:128 = odd sample of the pair.
    xb = x.astype(BF16)
    nb = x.shape[0]
    xp = np.zeros((nb, H, WP, C), dtype=BF16)
    xp[:, :, 1:W + 1, :] = xb
    flat = xp.reshape(nb, SP, C)
    xT = flat.transpose(0, 2, 1)                        # [B, C, SP]
    x2 = np.empty((nb // 2, 128, SP), dtype=BF16)
    x2[:, 0:C, :] = xT[0::2]
    x2[:, C:128, :] = xT[1::2]
    return np.ascontiguousarray(x2)


def _host_pack_wk(Wk):
    # [K, 3, 3, C, F] -> [128, K*9*F] fp32, tap-major per kernel, with
    # the channel rows duplicated on partitions 64:128 (sample B half).
    wt = np.transpose(Wk, (3, 0, 1, 2, 4))          # [C, K, kh, kw, F]
    w = wt.reshape(C, KK * TAPS * F)
    return np.ascontiguousarray(np.concatenate([w, w], axis=0))


def _host_inputs(inputs):
    """Shared host-side packing for kernel() and test harnesses."""
    x2 = _host_pack_x(np.asarray(inputs["x"]))
    wk_h = _host_pack_wk(np.asarray(inputs["Wk"]).astype(np.float32))
    w1 = (np.asarray(inputs["att_w1"]) / (H * W)).astype(np.float32)
    w1s = np.zeros((128, 64), dtype=np.float32)
    w1s[0:C, 0:HID] = w1
    w1s[C:128, 0:HID] = w1
    b1_h = np.ascontiguousarray(
        np.asarray(inputs["att_b1"]).reshape(HID, 1).astype(np.float32))
    w2s = np.zeros((64, KK), dtype=np.float32)
    w2s[0:HID, :] = (np.asarray(inputs["att_w2"]) / TEMP).astype(np.float32)
    b2_h = np.ascontiguousarray(
        (np.asarray(inputs["att_b2"]) / TEMP).reshape(1, KK)
        .astype(np.float32))
    bkt = np.transpose(np.asarray(inputs["bk"]), (1, 0)).astype(np.float32)
    bkt_h = np.ascontiguousarray(np.concatenate([bkt, bkt], axis=0))
    per_core = []
    for c in range(NCORES):
        per_core.append({
            "x2": x2[c * NPAIR:(c + 1) * NPAIR],
            "wk": wk_h, "w1s": w1s, "b1": b1_h,
            "w2s": w2s, "b2": b2_h, "bkt": bkt_h,
        })
    return per_core


def kernel(x, Wk, bk, att_w1, att_b1, att_w2, att_b2):
    from concourse import bass_utils

    nc = _get_program()
    in_maps = _host_inputs({
        "x": x, "Wk": Wk, "bk": bk, "att_w1": att_w1,
        "att_b1": att_b1, "att_w2": att_w2, "att_b2": att_b2,
    })
    res = bass_utils.run_bass_kernel_spmd(nc, in_maps,
                                          core_ids=list(range(NCORES)))

    y = np.empty((B, H, W, F), dtype=np.float32)
    for c in range(NCORES):
        yp = res.results[c]["ypad"]                 # [BPC, 128, HALF]
        arr = yp.reshape(BPC, 2, F, H // 2, WP)     # (b, half, f, row, col)
        y[c * BPC:(c + 1) * BPC] = (
            arr[:, :, :, :, 1:W + 1]
            .transpose(0, 1, 3, 4, 2)
            .reshape(BPC, H, W, F)
            .astype(np.float32))
    return y


# revision 18
# speedup vs baseline: 1.0097x; 1.0097x over previous
"""DynamicConv2D Trainium2 kernel (8-core SPMD, data-parallel over batch).

Per sample: GAP -> MLP -> softmax routing over K=4 kernel banks, weight-space
aggregation, then a 3x3 SAME conv with the per-sample aggregated kernel.

Device strategy (per core, 4 samples processed as 2 stacked PAIRS):
  - Host packs x TRANSPOSED and width-padded bf16 [128, SP] per PAIR:
    partitions 0:64 = sample A channels, 64:128 = sample B channels.
    No shifted copy -- input DMA bytes are halved vs the paired-tap layout.
  - Conv uses the PE's 64x64 quadrant tiling: 4 independent tiles
    (rowgrp = sample, colgrp = image half) stream concurrently, 9 K=64
    taps each, accumulating in PSUM.  That is ~100% MAC utilization
    (the K=128 paired-tap scheme wastes 1/3 on zero rows).
  - The PE queue serializes LDWEIGHTS at ~108 ns each, so each weight
    load is amortized over two N=512 matmuls ([128,1024] two-bank PSUM
    supergroups); a post-compile pass deletes the redundant LDWEIGHTS
    the legalizer inserts (hardware keeps tile weights across matmuls).
    The 8320-col half splits into 7x1024 + 1x1152(512+512+128); the 128
    tail rides the MLP's PSUM banks so it adds no extra weight loads.
  - Routing MLP: layer-1 for sample B uses tile (64,0) (SBUF 64:128 ->
    PSUM 0:64), so BOTH chains continue on partitions 0:64 and the
    softmax/pi broadcast is a plain partition-0 broadcast.  pi is
    normalized on-chip (1/sum folded before W_agg), drains are bias-only.
  - Engine split: SP queue = input loads; ACT = PSUM drains (+bias) and
    output stores; DVE = pooled partials + routing chain + W_agg FMAs;
    GpSimd = halo memsets + pi broadcast; PE = conv + MLP matmuls.
  - Software pipelining: routing chain for pair s+1 is emitted between
    the first supergroups of pair s's conv; loads run two pairs ahead.
"""

import numpy as np
import ml_dtypes

BF16 = ml_dtypes.bfloat16

B, H, W, C, F = 32, 128, 128, 64, 64
KK, HID = 4, 16
TAPS = 9
TEMP = 30.0
NCORES, BPC = 8, 4
NPAIR = BPC // 2    # 2 pairs per core
WP = W + 2          # padded width (zero col at w'=0 and w'=129)
SP = H * WP         # 16640 padded spatial per sample
PADL = 136          # SBUF zero halo before the image (taps read to -WP-1)
PADR = 136          # SBUF zero halo after (taps read up to +WP+1)
HALF = SP // 2      # 8320, image halves (rows <64 / >=64)
SPAD = PADL + SP + PADR     # 16912: halos live in DRAM (no memsets --
                            # a device-side halo memset picks up a WAR
                            # coarsened 2 pairs late and stalls the loads)
CHW = SPAD // 4     # input DMA chunk width (4228 cols, ~1.08 MB)
SF = TAPS * F       # 576 weight cols per sample half

# supergroups over one half: (col offset, [(block off, block width), ...])
SGS = [(g * 1024, [(0, 512), (512, 512)]) for g in range(7)]
SGS.append((7168, [(0, 512), (512, 512), (1024, 128)]))

_CACHE = {}


def _ldw_key(ins_ap):
    """Stable identity key for a lowered weights AP."""
    return repr(ins_ap)


def _dedup_ldweights(nc):
    """Remove InstLdweights that reload the exact weights already resident
    in the same PE tile.  The legalizer inserts one load per matmul; the
    hardware keeps a tile's stationary weights across matmuls, so a
    second load of the same AP into the same tile is pure queue time
    (~108 ns each).  Only sync-free loads are removed; any matmul whose
    weights AP differs from the tile's resident key invalidates it."""
    import concourse.mybir as mybir

    removed = 0
    for blk in nc.main_func.blocks:
        last = {}
        keep = []
        for ins in blk.instructions:
            if isinstance(ins, mybir.InstLdweights):
                tp = tuple(ins.tile_position or (0, 0))
                key = _ldw_key(ins.ins[0])
                si = getattr(ins, "sync_info", None)
                clean = si is None or (
                    not getattr(si, "on_wait", None)
                    and not getattr(si, "on_update", None)
                )
                if clean and last.get(tp) == key:
                    removed += 1
                    continue
                last[tp] = key
            elif isinstance(ins, mybir.InstMatmult):
                tp = tuple(ins.tile_position or (0, 0))
                wkey = _ldw_key(ins.ins[1])
                if last.get(tp) != wkey:
                    # self/alternate load (e.g. fp32 MLP matmul) clobbers
                    last[tp] = None
            keep.append(ins)
        blk.instructions[:] = keep
    return removed


def _build_program(dbg=False, reps=1):
    import concourse.bacc as bacc
    import concourse.mybir as mybir
    import concourse.tile as tile

    f32 = mybir.dt.float32
    bf16 = mybir.dt.bfloat16
    AX = mybir.AxisListType.X
    ALU = mybir.AluOpType

    nc = bacc.Bacc("TRN2", target_bir_lowering=False, debug=False)

    x2_d = nc.dram_tensor("x2", [NPAIR, 128, SPAD], bf16,
                          kind="ExternalInput")
    wk_d = nc.dram_tensor("wk", [128, KK * SF], f32, kind="ExternalInput")
    w1s_d = nc.dram_tensor("w1s", [128, 64], f32, kind="ExternalInput")
    b1_d = nc.dram_tensor("b1", [HID, 1], f32, kind="ExternalInput")
    w2s_d = nc.dram_tensor("w2s", [64, KK], f32, kind="ExternalInput")
    b2_d = nc.dram_tensor("b2", [1, KK], f32, kind="ExternalInput")
    bkt_d = nc.dram_tensor("bkt", [128, KK], f32, kind="ExternalInput")
    yp_d = nc.dram_tensor("ypad", [BPC, 128, HALF], bf16,
                          kind="ExternalOutput")
    if dbg:
        dpib_d = nc.dram_tensor("dpib", [128, 2 * KK], f32,
                                kind="ExternalOutput")
        dwg_d = nc.dram_tensor("dwg", [NPAIR, 128, SF], bf16,
                               kind="ExternalOutput")

    with tile.TileContext(nc) as tc:
        from contextlib import ExitStack
        with ExitStack() as ctx:
            # PSUM: four static [128,1024] two-bank tiles (all 8 banks).
            # A-supergroups ping-pong pc0/pc1, B-supergroups pc2/pc3, so
            # every conv bank has a full 2-supergroup drain window.  The
            # MLP tile and the 128-col tail borrow pc0 during its idle
            # windows (mid-sg1 / sg7).
            psp = ctx.enter_context(tc.tile_pool(name="psp", bufs=1,
                                                 space="PSUM"))
            cst = ctx.enter_context(tc.tile_pool(name="cst", bufs=1))
            xtp = ctx.enter_context(tc.tile_pool(name="xtp", bufs=4))
            ytp = ctx.enter_context(tc.tile_pool(name="ytp", bufs=3))
            wgp = ctx.enter_context(tc.tile_pool(name="wgp", bufs=2))
            smp = ctx.enter_context(tc.tile_pool(name="smp", bufs=2))

            wk_t = cst.tile([128, KK * SF], f32)
            w1s_t = cst.tile([128, 64], f32)
            b1_t = cst.tile([HID, 1], f32)
            w2s_t = cst.tile([64, KK], f32)
            b2_t = cst.tile([1, KK], f32)
            bkt_t = cst.tile([128, KK], f32)
            bagg_t = cst.tile([128, BPC], f32)
            hbuf_t = cst.tile([128, 128], f32)  # col 0 = hA, col 64 = hB

            def emit_consts():
                nc.gpsimd.memset(hbuf_t[:], 0.0)
                nc.sync.dma_start(w1s_t[:], w1s_d.ap())
                nc.sync.dma_start(b1_t[:], b1_d.ap())
                nc.sync.dma_start(w2s_t[:], w2s_d.ap())
                nc.sync.dma_start(b2_t[:], b2_d.ap())
                nc.sync.dma_start(wk_t[:], wk_d.ap())
                nc.sync.dma_start(bkt_t[:], bkt_d.ap())

            def emit_load(p):
                """Input chunk DMAs (halos included from DRAM) + pooled
                partials.  The reduces sit ADJACENT to their dma_starts
                so the framework binds their waits to the right chunk
                completions; the 4-deep xt ring means the chunks land a
                full pair early, so these reduces drain off the DVE
                queue before the next routing chain is emitted.  The
                halo zeros fold into the pooled sums harmlessly."""
                xt = xtp.tile([128, SPAD], bf16, tag="xt")
                pp = smp.tile([128, 4], f32, tag="pp")
                for i in range(4):
                    o = i * CHW
                    nc.sync.dma_start(xt[:, o:o + CHW],
                                      x2_d.ap()[p][:, o:o + CHW])
                    nc.vector.reduce_sum(pp[:, i:i + 1],
                                         xt[:, o:o + CHW], axis=AX)
                return xt, pp

            def emit_chain_mm(pp):
                """Routing part A: pooled combine -> MLP, in a 2-bank
                pc0 tile borrowed mid-sg1 (pc0 idles then).  hps/lps for
                A live in bank a, for B in bank b; sample B's layer-1
                uses tile (64,0) so both chains continue on partitions
                0:64."""
                pl = smp.tile([128, 1], f32, tag="pl")
                nc.vector.reduce_sum(pl[:], pp[:], axis=AX)
                mm = psp.tile([128, 1024], f32, tag="pc0")
                nc.tensor.matmul(mm[0:64, 0:1], lhsT=w1s_t[0:64, :],
                                 rhs=pl[0:64, 0:1], start=True, stop=True,
                                 tile_position=(0, 0))
                nc.tensor.matmul(mm[0:64, 512:513], lhsT=w1s_t[64:128, :],
                                 rhs=pl[64:128, 0:1], start=True, stop=True,
                                 tile_position=(64, 0))
                # relu(h + b1) into hbuf cols 0 (A) and 64 (B)
                nc.vector.tensor_scalar(hbuf_t[0:HID, 0:1],
                                        mm[0:HID, 0:1],
                                        scalar1=b1_t[:, 0:1], scalar2=0.0,
                                        op0=ALU.add, op1=ALU.max)
                nc.vector.tensor_scalar(hbuf_t[0:HID, 64:65],
                                        mm[0:HID, 512:513],
                                        scalar1=b1_t[:, 0:1], scalar2=0.0,
                                        op0=ALU.add, op1=ALU.max)
                nc.tensor.matmul(mm[0:64, 4:4 + KK],
                                 lhsT=hbuf_t[0:64, 0:64],
                                 rhs=w2s_t[:], start=True, stop=True,
                                 tile_position=(0, 0))
                nc.tensor.matmul(mm[0:64, 516:516 + KK],
                                 lhsT=hbuf_t[0:64, 64:128],
                                 rhs=w2s_t[:], start=True, stop=True,
                                 tile_position=(0, 0))
                return mm

            def emit_chain_post(s, mm):
                """Routing part B (DVE/GpSimd): softmax via 3rd-order
                Taylor exp (|logits/T| ~ 1e-3), normalized on-chip, then
                W_agg for both samples and the bias columns."""
                exb = smp.tile([1, 2 * KK], f32, tag="exb")
                for half in range(2):
                    lps = mm[0:1, 512 * half + 4:512 * half + 4 + KK]
                    lg = smp.tile([1, KK], f32, tag=f"lg{half}")
                    nc.vector.tensor_tensor(lg[:], lps, b2_t[:],
                                            op=ALU.add)
                    # exp(z) ~= 1 + z + z^2/2
                    e1 = smp.tile([1, KK], f32, tag=f"e1{half}")
                    nc.vector.tensor_scalar(e1[:], lg[:], scalar1=0.5,
                                            scalar2=1.0, op0=ALU.mult,
                                            op1=ALU.add)
                    nc.vector.tensor_tensor(e1[:], e1[:], lg[:],
                                            op=ALU.mult)
                    ex = smp.tile([1, KK], f32, tag=f"ex{half}")
                    nc.vector.tensor_scalar(ex[:], e1[:], scalar1=1.0,
                                            scalar2=None, op0=ALU.add)
                    sm = smp.tile([1, 1], f32, tag=f"sm{half}")
                    nc.vector.reduce_sum(sm[:], ex[:], axis=AX)
                    rc = smp.tile([1, 1], f32, tag=f"rc{half}")
                    nc.vector.reciprocal(rc[:], sm[:])
                    nc.vector.tensor_scalar(
                        exb[:, half * KK:(half + 1) * KK], ex[:],
                        scalar1=rc[0:1, 0:1], scalar2=None, op0=ALU.mult)
                pib = smp.tile([128, 2 * KK], f32, tag="pib")
                nc.gpsimd.partition_broadcast(pib[:], exb[:])
                # per-half pi columns: rows 0:64 <- A's pi, 64:128 <- B's
                pim = smp.tile([128, KK], f32, tag="pim")
                nc.vector.tensor_scalar(pim[0:64, :], pib[0:64, 0:KK],
                                        scalar1=0.0, scalar2=None,
                                        op0=ALU.add)
                nc.vector.tensor_scalar(pim[64:128, :], pib[64:128, KK:],
                                        scalar1=0.0, scalar2=None,
                                        op0=ALU.add)
                # W_agg = sum_k pi_k * Wk for both halves at once
                acc = wgp.tile([128, SF], f32, tag="acc")
                nc.vector.tensor_scalar(acc[:], wk_t[:, 0:SF],
                                        scalar1=pim[:, 0:1], scalar2=None,
                                        op0=ALU.mult)
                for k in range(1, KK - 1):
                    nc.vector.scalar_tensor_tensor(
                        acc[:], wk_t[:, k * SF:(k + 1) * SF],
                        pim[:, k:k + 1], acc[:], op0=ALU.mult, op1=ALU.add)
                wg = wgp.tile([128, SF], bf16, tag="wg")
                nc.vector.scalar_tensor_tensor(
                    wg[:], wk_t[:, (KK - 1) * SF:KK * SF],
                    pim[:, KK - 1:KK], acc[:], op0=ALU.mult, op1=ALU.add)
                # bias columns (pi already normalized): bagg[:, sample]
                for half in range(2):
                    ca = (2 * (s % NPAIR) + half)
                    bu = smp.tile([128, 1], f32, tag=f"bu{half}")
                    nc.vector.tensor_scalar(
                        bu[:], bkt_t[:, 0:1],
                        scalar1=pib[:, half * KK:half * KK + 1],
                        scalar2=None, op0=ALU.mult)
                    for k in range(1, KK - 1):
                        nc.vector.scalar_tensor_tensor(
                            bu[:], bkt_t[:, k:k + 1],
                            pib[:, half * KK + k:half * KK + k + 1], bu[:],
                            op0=ALU.mult, op1=ALU.add)
                    nc.vector.scalar_tensor_tensor(
                        bagg_t[:, ca:ca + 1], bkt_t[:, KK - 1:KK],
                        pib[:, half * KK + KK - 1:half * KK + KK], bu[:],
                        op0=ALU.mult, op1=ALU.add)
                if dbg:
                    nc.sync.dma_start(dpib_d.ap(), pib[:])
                    nc.sync.dma_start(dwg_d.ap()[s % NPAIR], wg[:])
                return wg

            def emit_sg(s, g, xt, wg, ytA, ytB, o, blocks, tap_hooks):
                """One supergroup: 9 taps x blocks, 4 quadrant tiles.
                Each tile's weights are loaded once per tap (the dedup
                pass removes the per-matmul reloads for blocks > 0).
                The 128-col tail (sg7 3rd block) accumulates in a
                borrowed pc0 tile: tailA in its bank a, tailB in bank b."""
                psA = psp.tile([128, 1024], f32, tag=f"pc{g % 2}")
                psB = psp.tile([128, 1024], f32, tag=f"pc{2 + g % 2}")
                tt = None
                if blocks[-1][0] == 1024:
                    tt = psp.tile([128, 1024], f32, tag="pc0")
                for j in range(TAPS):
                    off = (j // 3 - 1) * WP + (j % 3 - 1)
                    st = j == 0
                    sp = j == TAPS - 1
                    lA = wg[0:64, j * F:(j + 1) * F]
                    lB = wg[64:128, j * F:(j + 1) * F]
                    for (b, w) in blocks:
                        if b < 1024:
                            aA = psA[0:64, b:b + w]
                            aA2 = psA[64:128, b:b + w]
                            aB = psB[0:64, b:b + w]
                            aB2 = psB[64:128, b:b + w]
                        else:
                            aA = tt[0:64, 0:w]
                            aA2 = tt[64:128, 0:w]
                            aB = tt[0:64, 512:512 + w]
                            aB2 = tt[64:128, 512:512 + w]
                        base0 = PADL + o + b + off
                        base1 = base0 + HALF
                        nc.tensor.matmul(aA, lhsT=lA,
                                         rhs=xt[0:64, base0:base0 + w],
                                         start=st, stop=sp,
                                         tile_position=(0, 0))
                        nc.tensor.matmul(aA2, lhsT=lA,
                                         rhs=xt[0:64, base1:base1 + w],
                                         start=st, stop=sp,
                                         tile_position=(0, 64))
                        nc.tensor.matmul(aB, lhsT=lB,
                                         rhs=xt[64:128, base0:base0 + w],
                                         start=st, stop=sp,
                                         tile_position=(64, 0))
                        nc.tensor.matmul(aB2, lhsT=lB,
                                         rhs=xt[64:128, base1:base1 + w],
                                         start=st, stop=sp,
                                         tile_position=(64, 64))
                    hook = tap_hooks.get(j)
                    if hook is not None:
                        hook()
                gw = sum(w for (b, w) in blocks if b < 1024)
                caA = 2 * (s % NPAIR)
                ident = mybir.ActivationFunctionType.Identity
                if tt is not None:
                    # tail drains first: pc0 is the first tile the next
                    # pair's sg0 reuses
                    tw = blocks[-1][1]
                    to = o + blocks[-1][0]
                    nc.scalar.activation(ytA[:, to:to + tw], tt[:, 0:tw],
                                         ident,
                                         bias=bagg_t[:, caA:caA + 1])
                    nc.scalar.activation(ytB[:, to:to + tw],
                                         tt[:, 512:512 + tw], ident,
                                         bias=bagg_t[:, caA + 1:caA + 2])
                nc.scalar.activation(ytA[:, o:o + gw], psA[:, 0:gw], ident,
                                     bias=bagg_t[:, caA:caA + 1])
                nc.scalar.activation(ytB[:, o:o + gw], psB[:, 0:gw], ident,
                                     bias=bagg_t[:, caA + 1:caA + 2])

            def emit_conv(s, xt, wg, hooks, tap_hooks_by_sg):
                """Full conv for one pair; hooks[g] emitted after
                supergroup g, tap_hooks_by_sg[g][j] after tap j of
                supergroup g (pipelined routing for the next pair)."""
                ytA = ytp.tile([128, HALF], bf16, tag="yt")
                ytB = ytp.tile([128, HALF], bf16, tag="yt")
                for g, (o, blocks) in enumerate(SGS):
                    emit_sg(s, g, xt, wg, ytA, ytB, o, blocks,
                            tap_hooks_by_sg.get(g, {}))
                    hook = hooks.get(g)
                    if hook is not None:
                        hook()
                pr = s % NPAIR
                nc.scalar.dma_start(yp_d.ap()[2 * pr], ytA[:])
                nc.scalar.dma_start(yp_d.ap()[2 * pr + 1], ytB[:])

            # ---- software-pipelined main loop over pairs ----
            # per conv(s):  pre-sg0: DMA issue for pair s+2
            #               sg1 tap2/tap4: routing chain for pair s+1
            #               after sg5: pooled partials for pair s+2
            S = reps * NPAIR
            ld, ch = {}, {}
            ld[0] = emit_load(0)
            emit_consts()
            ch[0] = emit_chain_post(0, emit_chain_mm(ld[0][1]))
            if S > 1:
                ld[1] = emit_load(1)
            for s in range(S):
                if s + 2 < S:
                    ld[s + 2] = emit_load((s + 2) % NPAIR)
                xt, _ = ld.pop(s)
                wg = ch.pop(s)
                tap_hooks = {}
                if s + 1 < S:
                    box = {}

                    def hmm(box=box, s=s):
                        box["mm"] = emit_chain_mm(ld[s + 1][1])

                    def hpost(box=box, s=s):
                        ch[s + 1] = emit_chain_post(s + 1, box["mm"])

                    tap_hooks[1] = {2: hmm, 4: hpost}
                emit_conv(s, xt, wg, {}, tap_hooks)

    nc.compile()
    n = _dedup_ldweights(nc)
    assert n > 0, "LDWEIGHTS dedup removed nothing -- emission changed?"
    return nc


def _get_program():
    if "nc" not in _CACHE:
        _CACHE["nc"] = _build_program()
    return _CACHE["nc"]


def _host_pack_x(x):
    # [B, H, W, C] fp32 -> [B//2 pairs, 128, SP] bf16 per core slice:
    # width-padded, transposed to [c, spatial]; partitions 0:64 = even
    # sample, 64:128 = odd sample of the pair.
    xb = x.astype(BF16)
    nb = x.shape[0]
    xp = np.zeros((nb, H, WP, C), dtype=BF16)
    xp[:, :, 1:W + 1, :] = xb
    flat = xp.reshape(nb, SP, C)
    xT = flat.transpose(0, 2, 1)                        # [B, C, SP]
    x2 = np.zeros((nb // 2, 128, SPAD), dtype=BF16)
    x2[:, 0:C, PADL:PADL + SP] = xT[0::2]
    x2[:, C:128, PADL:PADL + SP] = xT[1::2]
    return np.ascontiguousarray(x2)


def _host_pack_wk(Wk):
    # [K, 3, 3, C, F] -> [128, K*9*F] fp32, tap-major per kernel, with
    # the channel rows duplicated on partitions 64:128 (sample B half).
    wt = np.transpose(Wk, (3, 0, 1, 2, 4))          # [C, K, kh, kw, F]
    w = wt.reshape(C, KK * TAPS * F)
    return np.ascontiguousarray(np.concatenate([w, w], axis=0))


def _host_inputs(inputs):
    """Shared host-side packing for kernel() and test harnesses."""
    x2 = _host_pack_x(np.asarray(inputs["x"]))
    wk_h = _host_pack_wk(np.asarray(inputs["Wk"]).astype(np.float32))
    w1 = (np.asarray(inputs["att_w1"]) / (H * W)).astype(np.float32)
    w1s = np.zeros((128, 64), dtype=np.float32)
    w1s[0:C, 0:HID] = w1
    w1s[C:128, 0:HID] = w1
    b1_h = np.ascontiguousarray(
        np.asarray(inputs["att_b1"]).reshape(HID, 1).astype(np.float32))
    w2s = np.zeros((64, KK), dtype=np.float32)
    w2s[0:HID, :] = (np.asarray(inputs["att_w2"]) / TEMP).astype(np.float32)
    b2_h = np.ascontiguousarray(
        (np.asarray(inputs["att_b2"]) / TEMP).reshape(1, KK)
        .astype(np.float32))
    bkt = np.transpose(np.asarray(inputs["bk"]), (1, 0)).astype(np.float32)
    bkt_h = np.ascontiguousarray(np.concatenate([bkt, bkt], axis=0))
    per_core = []
    for c in range(NCORES):
        per_core.append({
            "x2": x2[c * NPAIR:(c + 1) * NPAIR],
            "wk": wk_h, "w1s": w1s, "b1": b1_h,
            "w2s": w2s, "b2": b2_h, "bkt": bkt_h,
        })
    return per_core


def kernel(x, Wk, bk, att_w1, att_b1, att_w2, att_b2):
    from concourse import bass_utils

    nc = _get_program()
    in_maps = _host_inputs({
        "x": x, "Wk": Wk, "bk": bk, "att_w1": att_w1,
        "att_b1": att_b1, "att_w2": att_w2, "att_b2": att_b2,
    })
    res = bass_utils.run_bass_kernel_spmd(nc, in_maps,
                                          core_ids=list(range(NCORES)))

    y = np.empty((B, H, W, F), dtype=np.float32)
    for c in range(NCORES):
        yp = res.results[c]["ypad"]                 # [BPC, 128, HALF]
        arr = yp.reshape(BPC, 2, F, H // 2, WP)     # (b, half, f, row, col)
        y[c * BPC:(c + 1) * BPC] = (
            arr[:, :, :, :, 1:W + 1]
            .transpose(0, 1, 3, 4, 2)
            .reshape(BPC, H, W, F)
            .astype(np.float32))
    return y


# revision 24
# speedup vs baseline: 1.0229x; 1.0130x over previous
"""DynamicConv2D Trainium2 kernel (8-core SPMD, data-parallel over batch).

Per sample: GAP -> MLP -> softmax routing over K=4 kernel banks, weight-space
aggregation, then a 3x3 SAME conv with the per-sample aggregated kernel.

Device strategy (per core, 4 samples processed as 2 stacked PAIRS):
  - Host packs x TRANSPOSED and width-padded bf16 [128, SP] per PAIR:
    partitions 0:64 = sample A channels, 64:128 = sample B channels.
    No shifted copy -- input DMA bytes are halved vs the paired-tap layout.
  - Conv uses the PE's 64x64 quadrant tiling: 4 independent tiles
    (rowgrp = sample, colgrp = image half) stream concurrently, 9 K=64
    taps each, accumulating in PSUM.  That is ~100% MAC utilization
    (the K=128 paired-tap scheme wastes 1/3 on zero rows).
  - The PE queue serializes LDWEIGHTS at ~108 ns each, so each weight
    load is amortized over two N=512 matmuls ([128,1024] two-bank PSUM
    supergroups); a post-compile pass deletes the redundant LDWEIGHTS
    the legalizer inserts (hardware keeps tile weights across matmuls).
    The 8320-col half splits into 7x1024 + 1x1152(512+512+128); the 128
    tail rides the MLP's PSUM banks so it adds no extra weight loads.
  - Routing MLP: layer-1 for sample B uses tile (64,0) (SBUF 64:128 ->
    PSUM 0:64), so BOTH chains continue on partitions 0:64 and the
    softmax/pi broadcast is a plain partition-0 broadcast.  pi is
    normalized on-chip (1/sum folded before W_agg), drains are bias-only.
  - Engine split: SP queue = input loads; ACT = PSUM drains (+bias) and
    output stores; DVE = pooled partials + routing chain + W_agg FMAs;
    GpSimd = halo memsets + pi broadcast; PE = conv + MLP matmuls.
  - Software pipelining: routing chain for pair s+1 is emitted between
    the first supergroups of pair s's conv; loads run two pairs ahead.
"""

import numpy as np
import ml_dtypes

BF16 = ml_dtypes.bfloat16

B, H, W, C, F = 32, 128, 128, 64, 64
KK, HID = 4, 16
TAPS = 9
TEMP = 30.0
NCORES, BPC = 8, 4
NPAIR = BPC // 2    # 2 pairs per core
WP = W + 2          # padded width (zero col at w'=0 and w'=129)
SP = H * WP         # 16640 padded spatial per sample
PADL = 136          # SBUF zero halo before the image (taps read to -WP-1)
PADR = 136          # SBUF zero halo after (taps read up to +WP+1)
HALF = SP // 2      # 8320, image halves (rows <64 / >=64)
SPAD = PADL + SP + PADR     # 16912: halos live in DRAM (no memsets --
                            # a device-side halo memset picks up a WAR
                            # coarsened 2 pairs late and stalls the loads)
CHW = SPAD // 4     # input DMA chunk width (4228 cols, ~1.08 MB)
SF = TAPS * F       # 576 weight cols per sample half

# supergroups over one half: (col offset, [(block off, block width), ...])
SGS = [(g * 1024, [(0, 512), (512, 512)]) for g in range(7)]
SGS.append((7168, [(0, 512), (512, 512), (1024, 128)]))

_CACHE = {}


def _ldw_key(ins_ap):
    """Stable identity key for a lowered weights AP."""
    return repr(ins_ap)


def _dedup_ldweights(nc):
    """Remove InstLdweights that reload the exact weights already resident
    in the same PE tile.  The legalizer inserts one load per matmul; the
    hardware keeps a tile's stationary weights across matmuls, so a
    second load of the same AP into the same tile is pure queue time
    (~108 ns each).  Only sync-free loads are removed; any matmul whose
    weights AP differs from the tile's resident key invalidates it."""
    import concourse.mybir as mybir

    removed = 0
    for blk in nc.main_func.blocks:
        last = {}
        keep = []
        for ins in blk.instructions:
            if isinstance(ins, mybir.InstLdweights):
                tp = tuple(ins.tile_position or (0, 0))
                key = _ldw_key(ins.ins[0])
                si = getattr(ins, "sync_info", None)
                clean = si is None or (
                    not getattr(si, "on_wait", None)
                    and not getattr(si, "on_update", None)
                )
                if clean and last.get(tp) == key:
                    removed += 1
                    continue
                last[tp] = key
            elif isinstance(ins, mybir.InstMatmult):
                tp = tuple(ins.tile_position or (0, 0))
                wkey = _ldw_key(ins.ins[1])
                if last.get(tp) != wkey:
                    # self/alternate load (e.g. fp32 MLP matmul) clobbers
                    last[tp] = None
            keep.append(ins)
        blk.instructions[:] = keep
    return removed


def _build_program(dbg=False, reps=1):
    import concourse.bacc as bacc
    import concourse.mybir as mybir
    import concourse.tile as tile

    f32 = mybir.dt.float32
    bf16 = mybir.dt.bfloat16
    AX = mybir.AxisListType.X
    ALU = mybir.AluOpType

    nc = bacc.Bacc("TRN2", target_bir_lowering=False, debug=False)

    x2_d = nc.dram_tensor("x2", [NPAIR, 128, SPAD], bf16,
                          kind="ExternalInput")
    wk_d = nc.dram_tensor("wk", [128, KK * SF], f32, kind="ExternalInput")
    w1s_d = nc.dram_tensor("w1s", [128, 64], f32, kind="ExternalInput")
    b1_d = nc.dram_tensor("b1", [HID, 1], f32, kind="ExternalInput")
    w2s_d = nc.dram_tensor("w2s", [64, KK], f32, kind="ExternalInput")
    b2_d = nc.dram_tensor("b2", [1, KK], f32, kind="ExternalInput")
    bkt_d = nc.dram_tensor("bkt", [128, KK], f32, kind="ExternalInput")
    yp_d = nc.dram_tensor("ypad", [BPC, 128, HALF], bf16,
                          kind="ExternalOutput")
    if dbg:
        dpib_d = nc.dram_tensor("dpib", [128, 2 * KK], f32,
                                kind="ExternalOutput")
        dwg_d = nc.dram_tensor("dwg", [NPAIR, 128, SF], bf16,
                               kind="ExternalOutput")

    with tile.TileContext(nc) as tc:
        from contextlib import ExitStack
        with ExitStack() as ctx:
            # PSUM: four static [128,1024] two-bank tiles (all 8 banks).
            # A-supergroups ping-pong pc0/pc1, B-supergroups pc2/pc3, so
            # every conv bank has a full 2-supergroup drain window.  The
            # MLP tile and the 128-col tail borrow pc0 during its idle
            # windows (mid-sg1 / sg7).
            psp = ctx.enter_context(tc.tile_pool(name="psp", bufs=1,
                                                 space="PSUM"))
            cst = ctx.enter_context(tc.tile_pool(name="cst", bufs=1))
            xtp = ctx.enter_context(tc.tile_pool(name="xtp", bufs=4))
            ytp = ctx.enter_context(tc.tile_pool(name="ytp", bufs=3))
            wgp = ctx.enter_context(tc.tile_pool(name="wgp", bufs=2))
            smp = ctx.enter_context(tc.tile_pool(name="smp", bufs=2))

            wk_t = cst.tile([128, KK * SF], f32)
            w1s_t = cst.tile([128, 64], f32)
            b1_t = cst.tile([HID, 1], f32)
            w2s_t = cst.tile([64, KK], f32)
            b2_t = cst.tile([1, KK], f32)
            bkt_t = cst.tile([128, KK], f32)
            bagg_t = cst.tile([128, BPC], f32)
            hbuf_t = cst.tile([128, 128], f32)  # col 0 = hA, col 64 = hB
            ones_t = cst.tile([64, 64], f32)    # PE-broadcast stationary
            ebz_t = cst.tile([64, 2 * KK], f32)  # row 0 = (piA | piB)

            def emit_consts():
                nc.gpsimd.memset(hbuf_t[:], 0.0)
                nc.gpsimd.memset(ebz_t[:], 0.0)
                nc.gpsimd.memset(ones_t[:], 1.0)
                nc.sync.dma_start(w1s_t[:], w1s_d.ap())
                nc.sync.dma_start(b1_t[:], b1_d.ap())
                nc.sync.dma_start(w2s_t[:], w2s_d.ap())
                nc.sync.dma_start(b2_t[:], b2_d.ap())
                nc.sync.dma_start(wk_t[:], wk_d.ap())
                nc.sync.dma_start(bkt_t[:], bkt_d.ap())

            def emit_load(p):
                """Input chunk DMAs (halos included from DRAM) + pooled
                partials.  The reduces sit ADJACENT to their dma_starts
                so the framework binds their waits to the right chunk
                completions; the 4-deep xt ring means the chunks land a
                full pair early, so these reduces drain off the DVE
                queue before the next routing chain is emitted.  The
                halo zeros fold into the pooled sums harmlessly."""
                xt = xtp.tile([128, SPAD], bf16, tag="xt")
                for i in range(4):
                    o = i * CHW
                    nc.sync.dma_start(xt[:, o:o + CHW],
                                      x2_d.ap()[p][:, o:o + CHW])
                return xt

            def emit_pp(xt):
                """Pooled partials.  The framework binds these DMA waits
                one chunk-group late, so they release only mid-next-conv;
                emitted at the sg2 hook they queue BEHIND the routing
                chain's DVE ops and the late release is harmless."""
                pp = smp.tile([128, 4], f32, tag="pp")
                for i in range(4):
                    o = i * CHW
                    nc.vector.reduce_sum(pp[:, i:i + 1],
                                         xt[:, o:o + CHW], axis=AX)
                return pp

            def emit_chain_mm(pp):
                """Routing part A: pooled combine -> MLP, in a 2-bank
                pc0 tile borrowed mid-sg1 (pc0 idles then).  hps/lps for
                A live in bank a, for B in bank b; sample B's layer-1
                uses tile (64,0) so both chains continue on partitions
                0:64."""
                pl = smp.tile([128, 1], f32, tag="pl")
                nc.vector.reduce_sum(pl[:], pp[:], axis=AX)
                mm = psp.tile([128, 1024], f32, tag="pc0")
                nc.tensor.matmul(mm[0:64, 0:1], lhsT=w1s_t[0:64, :],
                                 rhs=pl[0:64, 0:1], start=True, stop=True,
                                 tile_position=(0, 0))
                nc.tensor.matmul(mm[0:64, 512:513], lhsT=w1s_t[64:128, :],
                                 rhs=pl[64:128, 0:1], start=True, stop=True,
                                 tile_position=(64, 0))
                # relu(h + b1) into hbuf cols 0 (A) and 64 (B)
                nc.vector.tensor_scalar(hbuf_t[0:HID, 0:1],
                                        mm[0:HID, 0:1],
                                        scalar1=b1_t[:, 0:1], scalar2=0.0,
                                        op0=ALU.add, op1=ALU.max)
                nc.vector.tensor_scalar(hbuf_t[0:HID, 64:65],
                                        mm[0:HID, 512:513],
                                        scalar1=b1_t[:, 0:1], scalar2=0.0,
                                        op0=ALU.add, op1=ALU.max)
                nc.tensor.matmul(mm[0:64, 4:4 + KK],
                                 lhsT=hbuf_t[0:64, 0:64],
                                 rhs=w2s_t[:], start=True, stop=True,
                                 tile_position=(0, 0))
                nc.tensor.matmul(mm[0:64, 516:516 + KK],
                                 lhsT=hbuf_t[0:64, 64:128],
                                 rhs=w2s_t[:], start=True, stop=True,
                                 tile_position=(0, 0))
                return mm

            def emit_chain_post(s, mm):
                """Routing part B (DVE/GpSimd): softmax via 3rd-order
                Taylor exp (|logits/T| ~ 1e-3), normalized on-chip, then
                W_agg for both samples and the bias columns."""
                for half in range(2):
                    lps = mm[0:1, 512 * half + 4:512 * half + 4 + KK]
                    lg = smp.tile([1, KK], f32, tag=f"lg{half}")
                    nc.vector.tensor_tensor(lg[:], lps, b2_t[:],
                                            op=ALU.add)
                    # exp(z) ~= 1 + z + z^2/2
                    e1 = smp.tile([1, KK], f32, tag=f"e1{half}")
                    nc.vector.tensor_scalar(e1[:], lg[:], scalar1=0.5,
                                            scalar2=1.0, op0=ALU.mult,
                                            op1=ALU.add)
                    nc.vector.tensor_tensor(e1[:], e1[:], lg[:],
                                            op=ALU.mult)
                    ex = smp.tile([1, KK], f32, tag=f"ex{half}")
                    nc.vector.tensor_scalar(ex[:], e1[:], scalar1=1.0,
                                            scalar2=None, op0=ALU.add)
                    sm = smp.tile([1, 1], f32, tag=f"sm{half}")
                    nc.vector.reduce_sum(sm[:], ex[:], axis=AX)
                    rc = smp.tile([1, 1], f32, tag=f"rc{half}")
                    nc.vector.reciprocal(rc[:], sm[:])
                    nc.vector.tensor_scalar(
                        ebz_t[0:1, half * KK:(half + 1) * KK], ex[:],
                        scalar1=rc[0:1, 0:1], scalar2=None, op0=ALU.mult)
                # broadcast (piA | piB) to all 128 partitions via the PE
                # (ones.T @ ebz; rows 1:63 of ebz are zero), landing in
                # the MLP tile's bank-a cols 8:16 -- keeps the routing
                # chain entirely off GpSimd.
                nc.tensor.matmul(mm[0:64, 8:8 + 2 * KK], lhsT=ones_t[:],
                                 rhs=ebz_t[:], start=True, stop=True,
                                 tile_position=(0, 0))
                nc.tensor.matmul(mm[64:128, 8:8 + 2 * KK], lhsT=ones_t[:],
                                 rhs=ebz_t[:], start=True, stop=True,
                                 tile_position=(0, 64))
                pib = smp.tile([128, 2 * KK], f32, tag="pib")
                nc.vector.tensor_scalar(pib[:], mm[:, 8:8 + 2 * KK],
                                        scalar1=0.0, scalar2=None,
                                        op0=ALU.add)
                # per-half pi columns: rows 0:64 <- A's pi, 64:128 <- B's
                pim = smp.tile([128, KK], f32, tag="pim")
                nc.vector.tensor_scalar(pim[0:64, :], pib[0:64, 0:KK],
                                        scalar1=0.0, scalar2=None,
                                        op0=ALU.add)
                nc.vector.tensor_scalar(pim[64:128, :], pib[64:128, KK:],
                                        scalar1=0.0, scalar2=None,
                                        op0=ALU.add)
                # W_agg = sum_k pi_k * Wk for both halves at once
                acc = wgp.tile([128, SF], f32, tag="acc")
                nc.vector.tensor_scalar(acc[:], wk_t[:, 0:SF],
                                        scalar1=pim[:, 0:1], scalar2=None,
                                        op0=ALU.mult)
                for k in range(1, KK - 1):
                    nc.vector.scalar_tensor_tensor(
                        acc[:], wk_t[:, k * SF:(k + 1) * SF],
                        pim[:, k:k + 1], acc[:], op0=ALU.mult, op1=ALU.add)
                wg = wgp.tile([128, SF], bf16, tag="wg")
                nc.vector.scalar_tensor_tensor(
                    wg[:], wk_t[:, (KK - 1) * SF:KK * SF],
                    pim[:, KK - 1:KK], acc[:], op0=ALU.mult, op1=ALU.add)
                # bias columns (pi already normalized): bagg[:, sample]
                for half in range(2):
                    ca = (2 * (s % NPAIR) + half)
                    bu = smp.tile([128, 1], f32, tag=f"bu{half}")
                    nc.vector.tensor_scalar(
                        bu[:], bkt_t[:, 0:1],
                        scalar1=pib[:, half * KK:half * KK + 1],
                        scalar2=None, op0=ALU.mult)
                    for k in range(1, KK - 1):
                        nc.vector.scalar_tensor_tensor(
                            bu[:], bkt_t[:, k:k + 1],
                            pib[:, half * KK + k:half * KK + k + 1], bu[:],
                            op0=ALU.mult, op1=ALU.add)
                    nc.vector.scalar_tensor_tensor(
                        bagg_t[:, ca:ca + 1], bkt_t[:, KK - 1:KK],
                        pib[:, half * KK + KK - 1:half * KK + KK], bu[:],
                        op0=ALU.mult, op1=ALU.add)
                if dbg:
                    nc.sync.dma_start(dpib_d.ap(), pib[:])
                    nc.sync.dma_start(dwg_d.ap()[s % NPAIR], wg[:])
                return wg

            def emit_sg(s, g, xt, wg, ytA, ytB, o, blocks, tap_hooks):
                """One supergroup: 9 taps x blocks, 4 quadrant tiles.
                Each tile's weights are loaded once per tap (the dedup
                pass removes the per-matmul reloads for blocks > 0).
                The 128-col tail (sg7 3rd block) accumulates in a
                borrowed pc0 tile: tailA in its bank a, tailB in bank b."""
                psA = psp.tile([128, 1024], f32, tag=f"pc{g % 2}")
                psB = psp.tile([128, 1024], f32, tag=f"pc{2 + g % 2}")
                tt = None
                if blocks[-1][0] == 1024:
                    tt = psp.tile([128, 1024], f32, tag="pc0")
                for j in range(TAPS):
                    off = (j // 3 - 1) * WP + (j % 3 - 1)
                    st = j == 0
                    sp = j == TAPS - 1
                    lA = wg[0:64, j * F:(j + 1) * F]
                    lB = wg[64:128, j * F:(j + 1) * F]
                    for (b, w) in blocks:
                        if b < 1024:
                            aA = psA[0:64, b:b + w]
                            aA2 = psA[64:128, b:b + w]
                            aB = psB[0:64, b:b + w]
                            aB2 = psB[64:128, b:b + w]
                        else:
                            aA = tt[0:64, 0:w]
                            aA2 = tt[64:128, 0:w]
                            aB = tt[0:64, 512:512 + w]
                            aB2 = tt[64:128, 512:512 + w]
                        base0 = PADL + o + b + off
                        base1 = base0 + HALF
                        nc.tensor.matmul(aA, lhsT=lA,
                                         rhs=xt[0:64, base0:base0 + w],
                                         start=st, stop=sp,
                                         tile_position=(0, 0))
                        nc.tensor.matmul(aA2, lhsT=lA,
                                         rhs=xt[0:64, base1:base1 + w],
                                         start=st, stop=sp,
                                         tile_position=(0, 64))
                        nc.tensor.matmul(aB, lhsT=lB,
                                         rhs=xt[64:128, base0:base0 + w],
                                         start=st, stop=sp,
                                         tile_position=(64, 0))
                        nc.tensor.matmul(aB2, lhsT=lB,
                                         rhs=xt[64:128, base1:base1 + w],
                                         start=st, stop=sp,
                                         tile_position=(64, 64))
                    hook = tap_hooks.get(j)
                    if hook is not None:
                        hook()
                gw = sum(w for (b, w) in blocks if b < 1024)
                caA = 2 * (s % NPAIR)
                ident = mybir.ActivationFunctionType.Identity
                if tt is not None:
                    # tail drains first: pc0 is the first tile the next
                    # pair's sg0 reuses
                    tw = blocks[-1][1]
                    to = o + blocks[-1][0]
                    nc.scalar.activation(ytA[:, to:to + tw], tt[:, 0:tw],
                                         ident,
                                         bias=bagg_t[:, caA:caA + 1])
                    nc.scalar.activation(ytB[:, to:to + tw],
                                         tt[:, 512:512 + tw], ident,
                                         bias=bagg_t[:, caA + 1:caA + 2])
                nc.scalar.activation(ytA[:, o:o + gw], psA[:, 0:gw], ident,
                                     bias=bagg_t[:, caA:caA + 1])
                nc.scalar.activation(ytB[:, o:o + gw], psB[:, 0:gw], ident,
                                     bias=bagg_t[:, caA + 1:caA + 2])

            def emit_conv(s, xt, wg, hooks, tap_hooks_by_sg):
                """Full conv for one pair; hooks[g] emitted after
                supergroup g, tap_hooks_by_sg[g][j] after tap j of
                supergroup g (pipelined routing for the next pair)."""
                ytA = ytp.tile([128, HALF], bf16, tag="yt")
                ytB = ytp.tile([128, HALF], bf16, tag="yt")
                for g, (o, blocks) in enumerate(SGS):
                    emit_sg(s, g, xt, wg, ytA, ytB, o, blocks,
                            tap_hooks_by_sg.get(g, {}))
                    hook = hooks.get(g)
                    if hook is not None:
                        hook()
                pr = s % NPAIR
                nc.scalar.dma_start(yp_d.ap()[2 * pr], ytA[:])
                nc.scalar.dma_start(yp_d.ap()[2 * pr + 1], ytB[:])

            # ---- software-pipelined main loop over pairs ----
            # per conv(s):  pre-sg0: DMA issue for pair s+2
            #               sg1 tap2/tap4: routing chain for pair s+1
            #               after sg5: pooled partials for pair s+2
            S = reps * NPAIR
            xts, pps, ch = {}, {}, {}
            xts[0] = emit_load(0)
            emit_consts()
            pps[0] = emit_pp(xts[0])
            ch[0] = emit_chain_post(0, emit_chain_mm(pps.pop(0)))
            if S > 1:
                xts[1] = emit_load(1)
                pps[1] = emit_pp(xts[1])
            for s in range(S):
                if s + 2 < S:
                    xts[s + 2] = emit_load((s + 2) % NPAIR)
                xt = xts.pop(s)
                wg = ch.pop(s)
                hooks = {}
                tap_hooks = {}
                if s + 1 < S:
                    box = {}

                    def hmm(box=box, s=s):
                        box["mm"] = emit_chain_mm(pps.pop(s + 1))

                    def hpost(box=box, s=s):
                        ch[s + 1] = emit_chain_post(s + 1, box["mm"])

                    tap_hooks[1] = {2: hmm, 6: hpost}
                if s + 2 < S:
                    def hpp(s=s):
                        pps[s + 2] = emit_pp(xts[s + 2])

                    hooks[2] = hpp
                emit_conv(s, xt, wg, hooks, tap_hooks)

    nc.compile()
    n = _dedup_ldweights(nc)
    assert n > 0, "LDWEIGHTS dedup removed nothing -- emission changed?"
    return nc


def _get_program():
    if "nc" not in _CACHE:
        _CACHE["nc"] = _build_program()
    return _CACHE["nc"]


def _host_pack_x(x):
    # [B, H, W, C] fp32 -> [B//2 pairs, 128, SP] bf16 per core slice:
    # width-padded, transposed to [c, spatial]; partitions 0:64 = even
    # sample, 64:128 = odd sample of the pair.
    xb = x.astype(BF16)
    nb = x.shape[0]
    xp = np.zeros((nb, H, WP, C), dtype=BF16)
    xp[:, :, 1:W + 1, :] = xb
    flat = xp.reshape(nb, SP, C)
    xT = flat.transpose(0, 2, 1)                        # [B, C, SP]
    x2 = np.zeros((nb // 2, 128, SPAD), dtype=BF16)
    x2[:, 0:C, PADL:PADL + SP] = xT[0::2]
    x2[:, C:128, PADL:PADL + SP] = xT[1::2]
    return np.ascontiguousarray(x2)


def _host_pack_wk(Wk):
    # [K, 3, 3, C, F] -> [128, K*9*F] fp32, tap-major per kernel, with
    # the channel rows duplicated on partitions 64:128 (sample B half).
    wt = np.transpose(Wk, (3, 0, 1, 2, 4))          # [C, K, kh, kw, F]
    w = wt.reshape(C, KK * TAPS * F)
    return np.ascontiguousarray(np.concatenate([w, w], axis=0))


def _host_inputs(inputs):
    """Shared host-side packing for kernel() and test harnesses."""
    x2 = _host_pack_x(np.asarray(inputs["x"]))
    wk_h = _host_pack_wk(np.asarray(inputs["Wk"]).astype(np.float32))
    w1 = (np.asarray(inputs["att_w1"]) / (H * W)).astype(np.float32)
    w1s = np.zeros((128, 64), dtype=np.float32)
    w1s[0:C, 0:HID] = w1
    w1s[C:128, 0:HID] = w1
    b1_h = np.ascontiguousarray(
        np.asarray(inputs["att_b1"]).reshape(HID, 1).astype(np.float32))
    w2s = np.zeros((64, KK), dtype=np.float32)
    w2s[0:HID, :] = (np.asarray(inputs["att_w2"]) / TEMP).astype(np.float32)
    b2_h = np.ascontiguousarray(
        (np.asarray(inputs["att_b2"]) / TEMP).reshape(1, KK)
        .astype(np.float32))
    bkt = np.transpose(np.asarray(inputs["bk"]), (1, 0)).astype(np.float32)
    bkt_h = np.ascontiguousarray(np.concatenate([bkt, bkt], axis=0))
    per_core = []
    for c in range(NCORES):
        per_core.append({
            "x2": x2[c * NPAIR:(c + 1) * NPAIR],
            "wk": wk_h, "w1s": w1s, "b1": b1_h,
            "w2s": w2s, "b2": b2_h, "bkt": bkt_h,
        })
    return per_core


def kernel(x, Wk, bk, att_w1, att_b1, att_w2, att_b2):
    from concourse import bass_utils

    nc = _get_program()
    in_maps = _host_inputs({
        "x": x, "Wk": Wk, "bk": bk, "att_w1": att_w1,
        "att_b1": att_b1, "att_w2": att_w2, "att_b2": att_b2,
    })
    res = bass_utils.run_bass_kernel_spmd(nc, in_maps,
                                          core_ids=list(range(NCORES)))

    y = np.empty((B, H, W, F), dtype=np.float32)
    for c in range(NCORES):
        yp = res.results[c]["ypad"]                 # [BPC, 128, HALF]
        arr = yp.reshape(BPC, 2, F, H // 2, WP)     # (b, half, f, row, col)
        y[c * BPC:(c + 1) * BPC] = (
            arr[:, :, :, :, 1:W + 1]
            .transpose(0, 1, 3, 4, 2)
            .reshape(BPC, H, W, F)
            .astype(np.float32))
    return y
